# revision 5
# baseline (speedup 1.0000x reference)
"""Trainium2 Bass kernel for nn_BDHGraphModel (gnn_message_passing).

Algorithm: the per-edge sparse recurrence is reformulated densely. Since
Gs == 1, duplicate edges share sigma dynamics, so sigma is carried as a
dense masked matrix Phi[s,d] = (C .* Sigma)/DECAY^k, where C is the edge
count matrix. Per layer-step:
    A   = x @ (C.*Sigma) = DECAY^k * (x @ Phi)
    y   = relu(A) @ GY          (GY[s,d] = sum of Gy over edges s->d)
    x'  = relu(y @ GX)
    Phi += DECAY^-(2t+1) * (C.*Gs/B) .* (y0^T @ x1)   (hebbian, layer 1)
Layer 1 skips y (x2 = relu(relu(A1) @ (GY@GX)) since y1 is never used by
the hebbian term). Readout logits = x2 @ W_ro^T + b_ro batched over T.

Sharding: d-columns of Phi/GY/GX/GYX split across 8 NeuronCores (256
cols each); activations allgathered between matmuls. All activations are
kept n-major ("transposed", [n, b]) so every matmul consumes the
constants as PE weights and produces partition-major shards.
"""
import os
import sys
from contextlib import ExitStack

import numpy as np

for p in ("/opt/trn_rl_repo", "/root/.axon_site/_ro/trn_rl_repo"):
    if os.path.isdir(p) and p not in sys.path:
        sys.path.append(p)

B, T_FULL, N, E, V = 8, 64, 2048, 65536, 32000
NCORES = 8
DECAY = 0.99
SH = N // NCORES // 2       # 128; d-shard = 256 = 2*128 cols per core
GQ = N // 128               # 16 K-chunks
VP = 4096                   # padded vocab shard per core (8*4096 = 32768)
NVT = VP // 128             # 32 v-tiles per core
NVC = 16                    # readout chunks (256 v each)

_T = int(os.environ.get("BDH_T", str(T_FULL)))


def _build_nc(T):
    import concourse.bass as bass  # noqa: F401
    import concourse.tile as tile
    from concourse import bacc, mybir

    f32 = mybir.dt.float32
    AF = mybir.ActivationFunctionType
    ALU = mybir.AluOpType

    nc = bacc.Bacc("TRN2", target_bir_lowering=False, debug=False,
                   num_devices=NCORES)

    xin = nc.dram_tensor("xin", [T, 128, GQ, B], f32, kind="ExternalInput")
    gy_d = nc.dram_tensor("gy", [128, GQ, 2 * SH], f32, kind="ExternalInput")
    gx_d = nc.dram_tensor("gx", [128, GQ, 2 * SH], f32, kind="ExternalInput")
    gyx_d = nc.dram_tensor("gyx", [128, GQ, 2 * SH], f32, kind="ExternalInput")
    cg_d = nc.dram_tensor("cg", [128, GQ, 2 * SH], f32, kind="ExternalInput")
    wro_d = nc.dram_tensor("wro", [NVC, 128, GQ, 256], f32, kind="ExternalInput")
    bro_d = nc.dram_tensor("bro", [128, NVT], f32, kind="ExternalInput")
    id_d = nc.dram_tensor("ident", [128, 128], f32, kind="ExternalInput")

    phi_out = nc.dram_tensor("phi_out", [128, GQ, 2 * SH], f32,
                             kind="ExternalOutput")
    lo_out = nc.dram_tensor("lo_out", [128, NVT, T * B], f32,
                            kind="ExternalOutput")

    # allgather bounce buffers (ping-pong to avoid cross-step races)
    agin = [nc.dram_tensor(f"agin{i}", [128, GQ], f32) for i in range(2)]
    agout = [nc.dram_tensor(f"agout{i}", [NCORES * 128, GQ], f32,
                            addr_space="Shared") for i in range(2)]
    rg = [list(range(NCORES))]
    ag_ctr = [0]

    with tile.TileContext(nc) as tc:
        with ExitStack() as ctx:
            const = ctx.enter_context(tc.tile_pool(name="const", bufs=1))
            act = ctx.enter_context(tc.tile_pool(name="act", bufs=3))
            upd = ctx.enter_context(tc.tile_pool(name="upd", bufs=2))
            rpool = ctx.enter_context(tc.tile_pool(name="rpool", bufs=2))
            psA = ctx.enter_context(tc.tile_pool(name="psA", bufs=2, space="PSUM"))
            psT = ctx.enter_context(tc.tile_pool(name="psT", bufs=2, space="PSUM"))
            psH = ctx.enter_context(tc.tile_pool(name="psH", bufs=2, space="PSUM"))
            psR = ctx.enter_context(tc.tile_pool(name="psR", bufs=2, space="PSUM"))

            gy = const.tile([128, GQ, 2 * SH], f32)
            gx = const.tile([128, GQ, 2 * SH], f32)
            gyx = const.tile([128, GQ, 2 * SH], f32)
            cg = const.tile([128, GQ, 2 * SH], f32)
            idn = const.tile([128, 128], f32)
            bro = const.tile([128, NVT], f32)
            nc.sync.dma_start(gy[:], gy_d[:])
            nc.sync.dma_start(gx[:], gx_d[:])
            nc.sync.dma_start(gyx[:], gyx_d[:])
            nc.sync.dma_start(cg[:], cg_d[:])
            nc.sync.dma_start(idn[:], id_d[:])
            nc.sync.dma_start(bro[:], bro_d[:])

            phi = [const.tile([128, GQ, 2 * SH], f32, tag=f"phi{i}",
                               name=f"phi{i}") for i in range(2)]
            nc.vector.memset(phi[0][:], 0.0)

            xhist = const.tile([128, T, GQ, B], f32)

            def shard_mm(lhsT, rhs, relu_scale=None):
                """[256,8] = lhsT[2048,256].T @ rhs[2048,8]; optional scaled
                relu; returns SBUF tile [128, 2, B]."""
                ps = psA.tile([128, 2, B], f32, tag="aps")
                for mt in range(2):
                    for g in range(GQ):
                        nc.tensor.matmul(
                            ps[:, mt, :], lhsT[:, g, 128 * mt:128 * (mt + 1)],
                            rhs[:, g, :], start=(g == 0), stop=(g == GQ - 1))
                out = act.tile([128, 2, B], f32, tag="own")
                if relu_scale is None:
                    nc.scalar.copy(out[:], ps[:])
                else:
                    nc.scalar.activation(out[:], ps[:], AF.Relu,
                                         scale=float(relu_scale))
                return out

            def allgather(own, dst_tile, dst_slice=None):
                """own [128,2,B] shard -> full [128,GQ,B] (or into given
                slice of a bigger tile)."""
                i = ag_ctr[0] % 2
                ag_ctr[0] += 1
                nc.sync.dma_start(agin[i][:], own[:].rearrange("p a b -> p (a b)"))
                nc.gpsimd.collective_compute(
                    "AllGather", mybir.AluOpType.bypass, replica_groups=rg,
                    ins=[agin[i][:]], outs=[agout[i][:]])
                dst = dst_tile[:] if dst_slice is None else dst_slice
                nc.sync.dma_start(
                    dst.rearrange("p (r gg) b -> p r gg b", r=NCORES),
                    agout[i][:].rearrange("(r p) (gg b) -> p r gg b",
                                          r=NCORES, b=B))

            for t in range(T):
                xt = act.tile([128, GQ, B], f32, tag="xt")
                nc.sync.dma_start(xt[:], xin[t])
                ph, phn = phi[t % 2], phi[(t + 1) % 2]

                # ---- layer 0 ----
                rA0 = shard_mm(ph, xt, relu_scale=DECAY ** (2 * t))
                rA0f = act.tile([128, GQ, B], f32, tag="rA0f")
                allgather(rA0, rA0f)

                y0 = shard_mm(gy, rA0f)
                y0f = act.tile([128, GQ, B], f32, tag="y0f")
                allgather(y0, y0f)

                x1 = shard_mm(gx, y0f, relu_scale=1.0)
                x1f = act.tile([128, GQ, B], f32, tag="x1f")
                allgather(x1, x1f)

                # ---- layer 1 ----
                rA1 = shard_mm(ph, x1f, relu_scale=DECAY ** (2 * t + 1))
                rA1f = act.tile([128, GQ, B], f32, tag="rA1f")
                allgather(rA1, rA1f)

                x2 = shard_mm(gyx, rA1f, relu_scale=1.0)
                allgather(x2, xhist, xhist[:, t, :, :])

                # ---- hebbian update: phn = ph + beta * cg .* (y0^T x1) ----
                y0bm = upd.tile([8, N], f32, tag="y0bm")
                for q in range(4):
                    tp = psT.tile([8, 512], f32, tag="tps")
                    for j in range(4):
                        g = 4 * q + j
                        nc.tensor.transpose(tp[:, 128 * j:128 * (j + 1)],
                                            y0f[:, g, :], idn[:])
                    nc.scalar.copy(y0bm[:, 512 * q:512 * (q + 1)], tp[:])
                x1bm = upd.tile([8, 2 * SH], f32, tag="x1bm")
                tp = psT.tile([8, 512], f32, tag="tps")
                for mt in range(2):
                    nc.tensor.transpose(tp[:, 128 * mt:128 * (mt + 1)],
                                        x1[:, mt, :], idn[:])
                nc.scalar.copy(x1bm[:], tp[:, 0:2 * SH])

                beta = float(DECAY ** (-(2 * t + 1)))
                for g in range(GQ):
                    hp = psH.tile([128, 2 * SH], f32, tag="hps")
                    nc.tensor.matmul(hp[:], y0bm[:8, 128 * g:128 * (g + 1)],
                                     x1bm[:8, :], start=True, stop=True)
                    t1 = upd.tile([128, 2 * SH], f32, tag="t1")
                    nc.vector.scalar_tensor_tensor(
                        t1[:], hp[:], beta, cg[:, g, :],
                        op0=ALU.mult, op1=ALU.mult)
                    nc.vector.tensor_add(phn[:, g, :], ph[:, g, :], t1[:])

            # ---- outputs ----
            nc.sync.dma_start(phi_out[:], phi[T % 2][:])

            for vc in range(NVC):
                wro_sb = rpool.tile([128, GQ, 256], f32, tag="wro")
                nc.sync.dma_start(wro_sb[:], wro_d[vc])
                for mt in range(2):
                    ps = psR.tile([128, T * B], f32, tag="rps")
                    for g in range(GQ):
                        nc.tensor.matmul(
                            ps[:], wro_sb[:, g, 128 * mt:128 * (mt + 1)],
                            xhist[:, :, g, :],
                            start=(g == 0), stop=(g == GQ - 1))
                    lo_sb = rpool.tile([128, T * B], f32, tag="lo")
                    nc.vector.tensor_scalar_add(
                        lo_sb[:], ps[:], bro[:, 2 * vc + mt:2 * vc + mt + 1])
                    nc.sync.dma_start(lo_out[:, 2 * vc + mt, :], lo_sb[:])

    nc.compile()
    return nc


def _preprocess(idx, edge_index, embedding, Gx, Gy, Gs, W_ro, b_ro, T):
    idx = np.asarray(idx)
    src = np.asarray(edge_index[0]).astype(np.int64)
    dst = np.asarray(edge_index[1]).astype(np.int64)
    key = src * N + dst
    Cm = np.zeros(N * N, np.float32)
    GYm = np.zeros(N * N, np.float32)
    GXm = np.zeros(N * N, np.float32)
    np.add.at(Cm, key, 1.0)
    np.add.at(GYm, key, np.asarray(Gy, np.float32))
    np.add.at(GXm, key, np.asarray(Gx, np.float32))
    GSm = np.zeros(N * N, np.float32)
    GSm[key] = np.asarray(Gs, np.float32)
    Cm = Cm.reshape(N, N)
    GYm = GYm.reshape(N, N)
    GXm = GXm.reshape(N, N)
    CG = (Cm * GSm.reshape(N, N)) / B
    GYX = (GYm @ GXm).astype(np.float32)

    X = np.asarray(embedding, np.float32)[idx]          # [B, T_full, N]
    X = X[:, :T, :]
    # xin[t, p, g, b] = X[b, t, g*128+p]
    xin = np.ascontiguousarray(
        X.reshape(B, T, GQ, 128).transpose(1, 3, 2, 0)).astype(np.float32)

    def shard_mat(M, c):
        # [128, GQ, 256]: dev[p, g, j] = M[g*128+p, 256c+j]
        Ms = M[:, 256 * c:256 * (c + 1)]
        return np.ascontiguousarray(
            Ms.reshape(GQ, 128, 256).transpose(1, 0, 2)).astype(np.float32)

    W = np.zeros((NCORES * VP, N), np.float32)
    W[:V] = np.asarray(W_ro, np.float32)
    bro_full = np.zeros(NCORES * VP, np.float32)
    bro_full[:V] = np.asarray(b_ro, np.float32)

    in_maps = []
    ident = np.eye(128, dtype=np.float32)
    for c in range(NCORES):
        Wc = W[VP * c:VP * (c + 1)]                      # [4096, 2048]
        # wro[vc, p, g, j] = Wc[512*vc + ..., g*128+p]; lhsT chunk layout:
        # columns m within chunk: j = 128*mt + m_in... flat 512 = v index
        WcT = Wc.T.reshape(GQ, 128, NVC, 256)            # [g, p, vc, 256]
        wro = np.ascontiguousarray(WcT.transpose(2, 1, 0, 3)).astype(np.float32)
        bro = np.ascontiguousarray(
            bro_full[VP * c:VP * (c + 1)].reshape(NVT, 128).T).astype(np.float32)
        in_maps.append(dict(
            xin=xin, gy=shard_mat(GYm, c), gx=shard_mat(GXm, c),
            gyx=shard_mat(GYX, c), cg=shard_mat(CG, c),
            wro=wro, bro=bro, ident=ident))
    return in_maps, Cm, src, dst


def kernel(idx, edge_index, embedding, Gx, Gy, Gs, W_ro, b_ro):
    from concourse.bass_utils import run_bass_kernel_spmd

    T = _T
    in_maps, Cm, src, dst = _preprocess(
        idx, edge_index, embedding, Gx, Gy, Gs, W_ro, b_ro, T)
    nc = _build_nc(T)
    res = run_bass_kernel_spmd(nc, in_maps, list(range(NCORES)))

    logits = np.zeros((B, T, NCORES * VP), np.float32)
    Phi = np.zeros((N, N), np.float32)
    for c in range(NCORES):
        lo = res.results[c]["lo_out"]                    # [128, NVT, T*B]
        # logits[b, t, VP*c + 128*mt + p] = lo[p, mt, t*B+b]
        lo = lo.reshape(128, NVT, T, B).transpose(3, 2, 1, 0)  # [B,T,NVT,128]
        logits[:, :, VP * c:VP * (c + 1)] = lo.reshape(B, T, VP)
        ph = res.results[c]["phi_out"]                   # [128, GQ, 256]
        Phi[:, 256 * c:256 * (c + 1)] = ph.transpose(1, 0, 2).reshape(N, 256)

    logits = logits[:, :, :V]
    sigma = (DECAY ** (2 * T)) * Phi[src, dst] / Cm[src, dst]
    return logits.astype(np.float32), sigma.astype(np.float32)


# revision 6
# speedup vs baseline: 1.0396x; 1.0396x over previous
"""Trainium2 Bass kernel for nn_BDHGraphModel (gnn_message_passing).

Algorithm: the per-edge sparse recurrence is reformulated densely. Since
Gs == 1, duplicate edges share sigma dynamics, so sigma is carried as a
dense masked matrix Phi[s,d] = (C .* Sigma)/DECAY^k, where C is the edge
count matrix. Per layer-step:
    A   = x @ (C.*Sigma) = DECAY^k * (x @ Phi)
    y   = relu(A) @ GY          (GY[s,d] = sum of Gy over edges s->d)
    x'  = relu(y @ GX)
    Phi += DECAY^-(2t+1) * (C.*Gs/B) .* (y0^T @ x1)   (hebbian, layer 1)
Layer 1 skips y (x2 = relu(relu(A1) @ (GY@GX)) since y1 is never used by
the hebbian term). Readout logits = x2 @ W_ro^T + b_ro batched over T.

Sharding: d-columns of Phi/GY/GX/GYX split across 8 NeuronCores (256
cols each); activations allgathered between matmuls. All activations are
kept n-major ("transposed", [n, b]) so every matmul consumes the
constants as PE weights and produces partition-major shards.
"""
import os
import sys
from contextlib import ExitStack

import numpy as np

for p in ("/opt/trn_rl_repo", "/root/.axon_site/_ro/trn_rl_repo"):
    if os.path.isdir(p) and p not in sys.path:
        sys.path.append(p)

B, T_FULL, N, E, V = 8, 64, 2048, 65536, 32000
NCORES = 8
DECAY = 0.99
SH = N // NCORES // 2       # 128; d-shard = 256 = 2*128 cols per core
GQ = N // 128               # 16 K-chunks
VP = 4096                   # padded vocab shard per core (8*4096 = 32768)
NVT = VP // 128             # 32 v-tiles per core
NVC = 16                    # readout chunks (256 v each)

_T = int(os.environ.get("BDH_T", str(T_FULL)))


def _build_nc(T):
    import concourse.bass as bass  # noqa: F401
    import concourse.tile as tile
    from concourse import bacc, mybir

    f32 = mybir.dt.float32
    AF = mybir.ActivationFunctionType
    ALU = mybir.AluOpType

    nc = bacc.Bacc("TRN2", target_bir_lowering=False, debug=False,
                   num_devices=NCORES)

    xin = nc.dram_tensor("xin", [T, 128, GQ, B], f32, kind="ExternalInput")
    gy_d = nc.dram_tensor("gy", [128, GQ, 2 * SH], f32, kind="ExternalInput")
    gx_d = nc.dram_tensor("gx", [128, GQ, 2 * SH], f32, kind="ExternalInput")
    gyx_d = nc.dram_tensor("gyx", [128, GQ, 2 * SH], f32, kind="ExternalInput")
    cg_d = nc.dram_tensor("cg", [128, GQ, 2 * SH], f32, kind="ExternalInput")
    wro_d = nc.dram_tensor("wro", [NVC, 128, GQ, 256], f32, kind="ExternalInput")
    bro_d = nc.dram_tensor("bro", [128, NVT], f32, kind="ExternalInput")
    id_d = nc.dram_tensor("ident", [128, 128], f32, kind="ExternalInput")
    phi0_d = nc.dram_tensor("phi0", [128, GQ, 2 * SH], f32, kind="ExternalInput")

    phi_out = nc.dram_tensor("phi_out", [128, GQ, 2 * SH], f32,
                             kind="ExternalOutput")
    lo_out = nc.dram_tensor("lo_out", [128, NVT, T * B], f32,
                            kind="ExternalOutput")

    # allgather bounce buffers (ping-pong to avoid cross-step races)
    agin = [nc.dram_tensor(f"agin{i}", [128, GQ], f32) for i in range(2)]
    agout = [nc.dram_tensor(f"agout{i}", [NCORES * 128, GQ], f32,
                            addr_space="Shared") for i in range(2)]
    rg = [list(range(NCORES))]
    ag_ctr = [0]

    with tile.TileContext(nc) as tc:
        with ExitStack() as ctx:
            const = ctx.enter_context(tc.tile_pool(name="const", bufs=1))
            act = ctx.enter_context(tc.tile_pool(name="act", bufs=3))
            upd = ctx.enter_context(tc.tile_pool(name="upd", bufs=2))
            rpool = ctx.enter_context(tc.tile_pool(name="rpool", bufs=2))
            psA = ctx.enter_context(tc.tile_pool(name="psA", bufs=2, space="PSUM"))
            psT = ctx.enter_context(tc.tile_pool(name="psT", bufs=2, space="PSUM"))
            psH = ctx.enter_context(tc.tile_pool(name="psH", bufs=2, space="PSUM"))
            psR = ctx.enter_context(tc.tile_pool(name="psR", bufs=2, space="PSUM"))

            gy = const.tile([128, GQ, 2 * SH], f32)
            gx = const.tile([128, GQ, 2 * SH], f32)
            gyx = const.tile([128, GQ, 2 * SH], f32)
            cg = const.tile([128, GQ, 2 * SH], f32)
            idn = const.tile([128, 128], f32)
            bro = const.tile([128, NVT], f32)
            nc.sync.dma_start(gy[:], gy_d[:])
            nc.sync.dma_start(gx[:], gx_d[:])
            nc.sync.dma_start(gyx[:], gyx_d[:])
            nc.sync.dma_start(cg[:], cg_d[:])
            nc.sync.dma_start(idn[:], id_d[:])
            nc.sync.dma_start(bro[:], bro_d[:])

            phi = [const.tile([128, GQ, 2 * SH], f32, tag=f"phi{i}",
                               name=f"phi{i}") for i in range(2)]
            nc.sync.dma_start(phi[0][:], phi0_d[:])

            xhist = const.tile([128, T, GQ, B], f32)

            def shard_mm(lhsT, rhs, relu_scale=None):
                """[256,8] = lhsT[2048,256].T @ rhs[2048,8]; optional scaled
                relu; returns SBUF tile [128, 2, B]."""
                ps = psA.tile([128, 2, B], f32, tag="aps")
                for mt in range(2):
                    for g in range(GQ):
                        nc.tensor.matmul(
                            ps[:, mt, :], lhsT[:, g, 128 * mt:128 * (mt + 1)],
                            rhs[:, g, :], start=(g == 0), stop=(g == GQ - 1))
                out = act.tile([128, 2, B], f32, tag="own")
                if relu_scale is None:
                    nc.scalar.copy(out[:], ps[:])
                else:
                    nc.scalar.activation(out[:], ps[:], AF.Relu,
                                         scale=float(relu_scale))
                return out

            def allgather(own, dst_tile, dst_slice=None):
                """own [128,2,B] shard -> full [128,GQ,B] (or into given
                slice of a bigger tile)."""
                i = ag_ctr[0] % 2
                ag_ctr[0] += 1
                nc.sync.dma_start(agin[i][:], own[:].rearrange("p a b -> p (a b)"))
                nc.gpsimd.collective_compute(
                    "AllGather", mybir.AluOpType.bypass, replica_groups=rg,
                    ins=[agin[i][:]], outs=[agout[i][:]])
                dst = dst_tile[:] if dst_slice is None else dst_slice
                nc.sync.dma_start(
                    dst.rearrange("p (r gg) b -> p r gg b", r=NCORES),
                    agout[i][:].rearrange("(r p) (gg b) -> p r gg b",
                                          r=NCORES, b=B))

            for t in range(T):
                xt = act.tile([128, GQ, B], f32, tag="xt")
                nc.sync.dma_start(xt[:], xin[t])
                ph, phn = phi[t % 2], phi[(t + 1) % 2]

                # ---- layer 0 ----
                rA0 = shard_mm(ph, xt, relu_scale=DECAY ** (2 * t))
                rA0f = act.tile([128, GQ, B], f32, tag="rA0f")
                allgather(rA0, rA0f)

                y0 = shard_mm(gy, rA0f)
                y0f = act.tile([128, GQ, B], f32, tag="y0f")
                allgather(y0, y0f)

                x1 = shard_mm(gx, y0f, relu_scale=1.0)
                x1f = act.tile([128, GQ, B], f32, tag="x1f")
                allgather(x1, x1f)

                # ---- layer 1 ----
                rA1 = shard_mm(ph, x1f, relu_scale=DECAY ** (2 * t + 1))
                rA1f = act.tile([128, GQ, B], f32, tag="rA1f")
                allgather(rA1, rA1f)

                x2 = shard_mm(gyx, rA1f, relu_scale=1.0)
                allgather(x2, xhist, xhist[:, t, :, :])

                # ---- hebbian update: phn = ph + beta * cg .* (y0^T x1) ----
                y0bm = upd.tile([8, N], f32, tag="y0bm")
                for q in range(4):
                    tp = psT.tile([8, 512], f32, tag="tps")
                    for j in range(4):
                        g = 4 * q + j
                        nc.tensor.transpose(tp[:, 128 * j:128 * (j + 1)],
                                            y0f[:, g, :], idn[:])
                    nc.scalar.copy(y0bm[:, 512 * q:512 * (q + 1)], tp[:])
                x1bm = upd.tile([8, 2 * SH], f32, tag="x1bm")
                tp = psT.tile([8, 512], f32, tag="tps")
                for mt in range(2):
                    nc.tensor.transpose(tp[:, 128 * mt:128 * (mt + 1)],
                                        x1[:, mt, :], idn[:])
                nc.scalar.copy(x1bm[:], tp[:, 0:2 * SH])

                beta = float(DECAY ** (-(2 * t + 1)))
                for g in range(GQ):
                    hp = psH.tile([128, 2 * SH], f32, tag="hps")
                    nc.tensor.matmul(hp[:], y0bm[:8, 128 * g:128 * (g + 1)],
                                     x1bm[:8, :], start=True, stop=True)
                    t1 = upd.tile([128, 2 * SH], f32, tag="t1")
                    nc.vector.scalar_tensor_tensor(
                        t1[:], hp[:], beta, cg[:, g, :],
                        op0=ALU.mult, op1=ALU.mult)
                    nc.vector.tensor_add(phn[:, g, :], ph[:, g, :], t1[:])

            # ---- outputs ----
            nc.sync.dma_start(phi_out[:], phi[T % 2][:])

            for vc in range(NVC):
                wro_sb = rpool.tile([128, GQ, 256], f32, tag="wro")
                nc.sync.dma_start(wro_sb[:], wro_d[vc])
                for mt in range(2):
                    ps = psR.tile([128, T * B], f32, tag="rps")
                    for g in range(GQ):
                        nc.tensor.matmul(
                            ps[:], wro_sb[:, g, 128 * mt:128 * (mt + 1)],
                            xhist[:, :, g, :],
                            start=(g == 0), stop=(g == GQ - 1))
                    lo_sb = rpool.tile([128, T * B], f32, tag="lo")
                    nc.vector.tensor_scalar_add(
                        lo_sb[:], ps[:], bro[:, 2 * vc + mt:2 * vc + mt + 1])
                    nc.sync.dma_start(lo_out[:, 2 * vc + mt, :], lo_sb[:])

    nc.compile()
    return nc


def _preprocess(idx, edge_index, embedding, Gx, Gy, Gs, W_ro, b_ro, T):
    idx = np.asarray(idx)
    src = np.asarray(edge_index[0]).astype(np.int64)
    dst = np.asarray(edge_index[1]).astype(np.int64)
    key = src * N + dst
    Cm = np.zeros(N * N, np.float32)
    GYm = np.zeros(N * N, np.float32)
    GXm = np.zeros(N * N, np.float32)
    np.add.at(Cm, key, 1.0)
    np.add.at(GYm, key, np.asarray(Gy, np.float32))
    np.add.at(GXm, key, np.asarray(Gx, np.float32))
    GSm = np.zeros(N * N, np.float32)
    GSm[key] = np.asarray(Gs, np.float32)
    Cm = Cm.reshape(N, N)
    GYm = GYm.reshape(N, N)
    GXm = GXm.reshape(N, N)
    CG = (Cm * GSm.reshape(N, N)) / B
    GYX = (GYm.astype(np.float64) @ GXm.astype(np.float64)).astype(np.float32)

    X = np.asarray(embedding, np.float32)[idx]          # [B, T_full, N]
    X = X[:, :T, :]
    # xin[t, p, g, b] = X[b, t, g*128+p]
    xin = np.ascontiguousarray(
        X.reshape(B, T, GQ, 128).transpose(1, 3, 2, 0)).astype(np.float32)

    def shard_mat(M, c):
        # [128, GQ, 256]: dev[p, g, j] = M[g*128+p, 256c+j]
        Ms = M[:, 256 * c:256 * (c + 1)]
        return np.ascontiguousarray(
            Ms.reshape(GQ, 128, 256).transpose(1, 0, 2)).astype(np.float32)

    W = np.zeros((NCORES * VP, N), np.float32)
    W[:V] = np.asarray(W_ro, np.float32)
    bro_full = np.zeros(NCORES * VP, np.float32)
    bro_full[:V] = np.asarray(b_ro, np.float32)

    in_maps = []
    ident = np.eye(128, dtype=np.float32)
    for c in range(NCORES):
        Wc = W[VP * c:VP * (c + 1)]                      # [4096, 2048]
        # wro[vc, p, g, j] = Wc[512*vc + ..., g*128+p]; lhsT chunk layout:
        # columns m within chunk: j = 128*mt + m_in... flat 512 = v index
        WcT = Wc.T.reshape(GQ, 128, NVC, 256)            # [g, p, vc, 256]
        wro = np.ascontiguousarray(WcT.transpose(2, 1, 0, 3)).astype(np.float32)
        bro = np.ascontiguousarray(
            bro_full[VP * c:VP * (c + 1)].reshape(NVT, 128).T).astype(np.float32)
        s0 = float(os.environ.get("BDH_SIGMA0", "0"))
        in_maps.append(dict(
            xin=xin, gy=shard_mat(GYm, c), gx=shard_mat(GXm, c),
            gyx=shard_mat(GYX, c), cg=shard_mat(CG, c),
            phi0=shard_mat(Cm * s0, c),
            wro=wro, bro=bro, ident=ident))
    return in_maps, Cm, src, dst


def kernel(idx, edge_index, embedding, Gx, Gy, Gs, W_ro, b_ro):
    from concourse.bass_utils import run_bass_kernel_spmd

    T = _T
    in_maps, Cm, src, dst = _preprocess(
        idx, edge_index, embedding, Gx, Gy, Gs, W_ro, b_ro, T)
    nc = _build_nc(T)
    res = run_bass_kernel_spmd(nc, in_maps, list(range(NCORES)))

    logits = np.zeros((B, T, NCORES * VP), np.float32)
    Phi = np.zeros((N, N), np.float32)
    for c in range(NCORES):
        lo = res.results[c]["lo_out"]                    # [128, NVT, T*B]
        # logits[b, t, VP*c + 128*mt + p] = lo[p, mt, t*B+b]
        lo = lo.reshape(128, NVT, T, B).transpose(3, 2, 1, 0)  # [B,T,NVT,128]
        logits[:, :, VP * c:VP * (c + 1)] = lo.reshape(B, T, VP)
        ph = res.results[c]["phi_out"]                   # [128, GQ, 256]
        Phi[:, 256 * c:256 * (c + 1)] = ph.transpose(1, 0, 2).reshape(N, 256)

    logits = logits[:, :, :V]
    sigma = (DECAY ** (2 * T)) * Phi[src, dst] / Cm[src, dst]
    return logits.astype(np.float32), sigma.astype(np.float32)


# revision 7
# speedup vs baseline: 1.0450x; 1.0052x over previous
"""Trainium2 Bass kernel for nn_BDHGraphModel (gnn_message_passing).

Algorithm: the per-edge sparse recurrence is reformulated densely. Since
Gs == 1, duplicate edges share sigma dynamics, so sigma is carried as a
dense masked matrix Phi[s,d] = (C .* Sigma)/DECAY^k, where C is the edge
count matrix. Per layer-step:
    A   = x @ (C.*Sigma) = DECAY^k * (x @ Phi)
    y   = relu(A) @ GY          (GY[s,d] = sum of Gy over edges s->d)
    x'  = relu(y @ GX)
    Phi += DECAY^-(2t+1) * (C.*Gs/B) .* (y0^T @ x1)   (hebbian, layer 1)
Layer 1 skips y (x2 = relu(relu(A1) @ (GY@GX)) since y1 is never used by
the hebbian term). Readout logits = x2 @ W_ro^T + b_ro batched over T.

Sharding: d-columns of Phi/GY/GX/GYX split across 8 NeuronCores (256
cols each); activations allgathered between matmuls. All activations are
kept n-major ("transposed", [n, b]) so every matmul consumes the
constants as PE weights and produces partition-major shards.
"""
import os
import sys
from contextlib import ExitStack

import numpy as np

for p in ("/opt/trn_rl_repo", "/root/.axon_site/_ro/trn_rl_repo"):
    if os.path.isdir(p) and p not in sys.path:
        sys.path.append(p)

B, T_FULL, N, E, V = 8, 64, 2048, 65536, 32000
NCORES = 8
DECAY = 0.99
SH = N // NCORES // 2       # 128; d-shard = 256 = 2*128 cols per core
GQ = N // 128               # 16 K-chunks
VP = 4096                   # padded vocab shard per core (8*4096 = 32768)
NVT = VP // 128             # 32 v-tiles per core
NVC = 16                    # readout chunks (256 v each)

_T = int(os.environ.get("BDH_T", str(T_FULL)))


def _build_nc(T):
    import concourse.bass as bass  # noqa: F401
    import concourse.tile as tile
    from concourse import bacc, mybir

    f32 = mybir.dt.float32
    AF = mybir.ActivationFunctionType
    ALU = mybir.AluOpType

    nc = bacc.Bacc("TRN2", target_bir_lowering=False, debug=False,
                   num_devices=NCORES)

    xin = nc.dram_tensor("xin", [T, 128, GQ, B], f32, kind="ExternalInput")
    gy_d = nc.dram_tensor("gy", [128, GQ, 2 * SH], f32, kind="ExternalInput")
    gyx_d = nc.dram_tensor("gyx", [128, GQ, 2 * SH], f32, kind="ExternalInput")
    cg_d = nc.dram_tensor("cg", [128, GQ, 2 * SH], f32, kind="ExternalInput")
    wro_d = nc.dram_tensor("wro", [NVC, 128, GQ, 256], f32, kind="ExternalInput")
    bro_d = nc.dram_tensor("bro", [128, NVT], f32, kind="ExternalInput")
    id_d = nc.dram_tensor("ident", [128, 128], f32, kind="ExternalInput")
    phi0_d = nc.dram_tensor("phi0", [128, GQ, 2 * SH], f32, kind="ExternalInput")

    phi_out = nc.dram_tensor("phi_out", [128, GQ, 2 * SH], f32,
                             kind="ExternalOutput")
    lo_out = nc.dram_tensor("lo_out", [128, NVT, T * B], f32,
                            kind="ExternalOutput")

    # allgather bounce buffers (ping-pong to avoid cross-step races)
    agin = [nc.dram_tensor(f"agin{i}", [128, GQ], f32) for i in range(4)]
    agout = [nc.dram_tensor(f"agout{i}", [NCORES * 128, GQ], f32,
                            addr_space="Shared") for i in range(4)]
    rg = [list(range(NCORES))]
    ag_ctr = [0]

    with tile.TileContext(nc) as tc:
        with ExitStack() as ctx:
            const = ctx.enter_context(tc.tile_pool(name="const", bufs=1))
            act = ctx.enter_context(tc.tile_pool(name="act", bufs=3))
            upd = ctx.enter_context(tc.tile_pool(name="upd", bufs=2))
            rpool = ctx.enter_context(tc.tile_pool(name="rpool", bufs=2))
            psA = ctx.enter_context(tc.tile_pool(name="psA", bufs=2, space="PSUM"))
            psT = ctx.enter_context(tc.tile_pool(name="psT", bufs=2, space="PSUM"))
            psH = ctx.enter_context(tc.tile_pool(name="psH", bufs=2, space="PSUM"))
            psR = ctx.enter_context(tc.tile_pool(name="psR", bufs=2, space="PSUM"))

            gy = const.tile([128, GQ, 2 * SH], f32)
            gyx = const.tile([128, GQ, 2 * SH], f32)
            cg = const.tile([128, GQ, 2 * SH], f32)
            idn = const.tile([128, 128], f32)
            bro = const.tile([128, NVT], f32)
            nc.sync.dma_start(gy[:], gy_d[:])
            nc.sync.dma_start(gyx[:], gyx_d[:])
            nc.sync.dma_start(cg[:], cg_d[:])
            nc.sync.dma_start(idn[:], id_d[:])
            nc.sync.dma_start(bro[:], bro_d[:])

            phi = [const.tile([128, GQ, 2 * SH], f32, tag=f"phi{i}",
                               name=f"phi{i}") for i in range(2)]
            nc.sync.dma_start(phi[0][:], phi0_d[:])

            xhist = const.tile([128, T, GQ, B], f32)

            def shard_mm(lhsT, rhs, relu_scale=None):
                """[256,8] = lhsT[2048,256].T @ rhs[2048,8]; optional scaled
                relu; returns SBUF tile [128, 2, B]."""
                ps = psA.tile([128, 2, B], f32, tag="aps")
                for mt in range(2):
                    for g in range(GQ):
                        nc.tensor.matmul(
                            ps[:, mt, :], lhsT[:, g, 128 * mt:128 * (mt + 1)],
                            rhs[:, g, :], start=(g == 0), stop=(g == GQ - 1))
                out = act.tile([128, 2, B], f32, tag="own")
                if relu_scale is None:
                    nc.scalar.copy(out[:], ps[:])
                else:
                    nc.scalar.activation(out[:], ps[:], AF.Relu,
                                         scale=float(relu_scale))
                return out

            def allgather(own, dst_tile, dst_slice=None):
                """own [128,2,B] shard -> full [128,GQ,B] (or into given
                slice of a bigger tile)."""
                i = ag_ctr[0] % 4
                ag_ctr[0] += 1
                nc.sync.dma_start(agin[i][:], own[:].rearrange("p a b -> p (a b)"))
                nc.gpsimd.collective_compute(
                    "AllGather", mybir.AluOpType.bypass, replica_groups=rg,
                    ins=[agin[i][:]], outs=[agout[i][:]])
                dst = dst_tile[:] if dst_slice is None else dst_slice
                nc.sync.dma_start(
                    dst.rearrange("p (r gg) b -> p r gg b", r=NCORES),
                    agout[i][:].rearrange("(r p) (gg b) -> p r gg b",
                                          r=NCORES, b=B))

            for t in range(T):
                xt = act.tile([128, GQ, B], f32, tag="xt")
                nc.sync.dma_start(xt[:], xin[t])
                ph, phn = phi[t % 2], phi[(t + 1) % 2]

                # ---- layer 0: A0, then recurrence branch first ----
                rA0 = shard_mm(ph, xt, relu_scale=DECAY ** (2 * t))
                rA0f = act.tile([128, GQ, B], f32, tag="rA0f")
                allgather(rA0, rA0f)

                # hebbian needs y0 (full, b-major) and x1 (own shard only).
                y0 = shard_mm(gy, rA0f)
                y0f = act.tile([128, GQ, B], f32, tag="y0f")
                allgather(y0, y0f)

                # x1 = relu(y0 @ GX) = relu(relu(A0) @ GYX) directly
                x1 = shard_mm(gyx, rA0f, relu_scale=1.0)

                y0bm = upd.tile([8, N], f32, tag="y0bm")
                for q in range(4):
                    tp = psT.tile([8, 512], f32, tag="tps")
                    for j in range(4):
                        g = 4 * q + j
                        nc.tensor.transpose(tp[:, 128 * j:128 * (j + 1)],
                                            y0f[:, g, :], idn[:])
                    nc.scalar.copy(y0bm[:, 512 * q:512 * (q + 1)], tp[:])
                x1bm = upd.tile([8, 2 * SH], f32, tag="x1bm")
                tp = psT.tile([8, 512], f32, tag="tps")
                for mt in range(2):
                    nc.tensor.transpose(tp[:, 128 * mt:128 * (mt + 1)],
                                        x1[:, mt, :], idn[:])
                nc.scalar.copy(x1bm[:], tp[:, 0:2 * SH])

                # phn = ph + beta * cg .* (y0^T x1)  -- the recurrence
                beta = float(DECAY ** (-(2 * t + 1)))
                for g in range(GQ):
                    hp = psH.tile([128, 2 * SH], f32, tag="hps")
                    nc.tensor.matmul(hp[:], y0bm[:8, 128 * g:128 * (g + 1)],
                                     x1bm[:8, :], start=True, stop=True)
                    t1 = upd.tile([128, 2 * SH], f32, tag="t1")
                    nc.vector.scalar_tensor_tensor(
                        t1[:], hp[:], beta, cg[:, g, :],
                        op0=ALU.mult, op1=ALU.mult)
                    nc.vector.tensor_add(phn[:, g, :], ph[:, g, :], t1[:])

                # ---- layer 1 (logits path, off the recurrence) ----
                x1f = act.tile([128, GQ, B], f32, tag="x1f")
                allgather(x1, x1f)

                rA1 = shard_mm(ph, x1f, relu_scale=DECAY ** (2 * t + 1))
                rA1f = act.tile([128, GQ, B], f32, tag="rA1f")
                allgather(rA1, rA1f)

                x2 = shard_mm(gyx, rA1f, relu_scale=1.0)
                allgather(x2, xhist, xhist[:, t, :, :])

            # ---- outputs ----
            nc.sync.dma_start(phi_out[:], phi[T % 2][:])

            for vc in range(NVC):
                wro_sb = rpool.tile([128, GQ, 256], f32, tag="wro")
                nc.sync.dma_start(wro_sb[:], wro_d[vc])
                for mt in range(2):
                    ps = psR.tile([128, T * B], f32, tag="rps")
                    for g in range(GQ):
                        nc.tensor.matmul(
                            ps[:], wro_sb[:, g, 128 * mt:128 * (mt + 1)],
                            xhist[:, :, g, :],
                            start=(g == 0), stop=(g == GQ - 1))
                    lo_sb = rpool.tile([128, T * B], f32, tag="lo")
                    nc.vector.tensor_scalar_add(
                        lo_sb[:], ps[:], bro[:, 2 * vc + mt:2 * vc + mt + 1])
                    nc.sync.dma_start(lo_out[:, 2 * vc + mt, :], lo_sb[:])

    nc.compile()
    return nc


def _preprocess(idx, edge_index, embedding, Gx, Gy, Gs, W_ro, b_ro, T):
    idx = np.asarray(idx)
    src = np.asarray(edge_index[0]).astype(np.int64)
    dst = np.asarray(edge_index[1]).astype(np.int64)
    key = src * N + dst
    Cm = np.zeros(N * N, np.float32)
    GYm = np.zeros(N * N, np.float32)
    GXm = np.zeros(N * N, np.float32)
    np.add.at(Cm, key, 1.0)
    np.add.at(GYm, key, np.asarray(Gy, np.float32))
    np.add.at(GXm, key, np.asarray(Gx, np.float32))
    GSm = np.zeros(N * N, np.float32)
    GSm[key] = np.asarray(Gs, np.float32)
    Cm = Cm.reshape(N, N)
    GYm = GYm.reshape(N, N)
    GXm = GXm.reshape(N, N)
    CG = (Cm * GSm.reshape(N, N)) / B
    GYX = (GYm.astype(np.float64) @ GXm.astype(np.float64)).astype(np.float32)

    X = np.asarray(embedding, np.float32)[idx]          # [B, T_full, N]
    X = X[:, :T, :]
    # xin[t, p, g, b] = X[b, t, g*128+p]
    xin = np.ascontiguousarray(
        X.reshape(B, T, GQ, 128).transpose(1, 3, 2, 0)).astype(np.float32)

    def shard_mat(M, c):
        # [128, GQ, 256]: dev[p, g, j] = M[g*128+p, 256c+j]
        Ms = M[:, 256 * c:256 * (c + 1)]
        return np.ascontiguousarray(
            Ms.reshape(GQ, 128, 256).transpose(1, 0, 2)).astype(np.float32)

    W = np.zeros((NCORES * VP, N), np.float32)
    W[:V] = np.asarray(W_ro, np.float32)
    bro_full = np.zeros(NCORES * VP, np.float32)
    bro_full[:V] = np.asarray(b_ro, np.float32)

    in_maps = []
    ident = np.eye(128, dtype=np.float32)
    for c in range(NCORES):
        Wc = W[VP * c:VP * (c + 1)]                      # [4096, 2048]
        # wro[vc, p, g, j] = Wc[512*vc + ..., g*128+p]; lhsT chunk layout:
        # columns m within chunk: j = 128*mt + m_in... flat 512 = v index
        WcT = Wc.T.reshape(GQ, 128, NVC, 256)            # [g, p, vc, 256]
        wro = np.ascontiguousarray(WcT.transpose(2, 1, 0, 3)).astype(np.float32)
        bro = np.ascontiguousarray(
            bro_full[VP * c:VP * (c + 1)].reshape(NVT, 128).T).astype(np.float32)
        s0 = float(os.environ.get("BDH_SIGMA0", "0"))
        in_maps.append(dict(
            xin=xin, gy=shard_mat(GYm, c),
            gyx=shard_mat(GYX, c), cg=shard_mat(CG, c),
            phi0=shard_mat(Cm * s0, c),
            wro=wro, bro=bro, ident=ident))
    return in_maps, Cm, src, dst


def kernel(idx, edge_index, embedding, Gx, Gy, Gs, W_ro, b_ro):
    from concourse.bass_utils import run_bass_kernel_spmd

    T = _T
    in_maps, Cm, src, dst = _preprocess(
        idx, edge_index, embedding, Gx, Gy, Gs, W_ro, b_ro, T)
    nc = _build_nc(T)
    res = run_bass_kernel_spmd(nc, in_maps, list(range(NCORES)))

    logits = np.zeros((B, T, NCORES * VP), np.float32)
    Phi = np.zeros((N, N), np.float32)
    for c in range(NCORES):
        lo = res.results[c]["lo_out"]                    # [128, NVT, T*B]
        # logits[b, t, VP*c + 128*mt + p] = lo[p, mt, t*B+b]
        lo = lo.reshape(128, NVT, T, B).transpose(3, 2, 1, 0)  # [B,T,NVT,128]
        logits[:, :, VP * c:VP * (c + 1)] = lo.reshape(B, T, VP)
        ph = res.results[c]["phi_out"]                   # [128, GQ, 256]
        Phi[:, 256 * c:256 * (c + 1)] = ph.transpose(1, 0, 2).reshape(N, 256)

    logits = logits[:, :, :V]
    sigma = (DECAY ** (2 * T)) * Phi[src, dst] / Cm[src, dst]
    return logits.astype(np.float32), sigma.astype(np.float32)
